# revision 1
# baseline (speedup 1.0000x reference)
"""Trainium2 Bass kernel for BERT4ETH adjacency build:
    data = values * (features @ a0_weight[0])        # [E]
    out  = segment_sum(data, rows, num_segments=3M)  # [3M]

Distribution strategy (8 NeuronCores): shard by OUTPUT node range.
Each core owns a contiguous range of 376832 nodes (23 groups x 16384
nodes).  The host-side shard step routes each edge to the core/group
that owns its destination node (a coarse 184-bucket assignment); all
per-edge arithmetic -- the feature dot products, index decomposition,
one-hot construction, and the scatter-accumulate itself -- runs on
device.  Because output ranges are disjoint there is no all-reduce;
the host just concatenates the 8 per-core outputs.

Device algorithm, per core:
  for each of 23 groups (16384 nodes each, edges pre-bucketed):
    d = values * sum_f w_f * feat_f          (DVE, dense)
    p = m & 127 ; c = m >> 7                 (m = node idx within group)
    for each 128-edge tile:
      W[k,i] = (iota_i == p_k) * d_k         (one tensor_scalar, 4x mode, bf16)
      X[k,j] = (iota_j == c_k)               (one tensor_scalar, 4x mode, bf16)
      psum[c,p] += X^T @ W                   (TensorE, f32 accumulate)
    copy psum -> accum[:, g*128:(g+1)*128]
  one DMA: accum -> out  (out[g,c,p] = node g*16384 + c*128 + p)

Note: walrus embeds at most ONE sync-wait in a DVE/PE instruction, so the
structure below is arranged (primer ops + explicit order deps) so that every
compute instruction depends on at most one unobserved semaphore.
"""

import numpy as np
import ml_dtypes

import concourse.bass as bass
import concourse.mybir as mybir
from concourse.bass_utils import run_bass_kernel_spmd

F32 = mybir.dt.float32
BF16 = mybir.dt.bfloat16
I32 = mybir.dt.int32

N_CORES = 8
NUM_NODES = 3_000_000
GROUP_NODES = 16384          # nodes per group = 128*128 psum bins
N_GROUPS = 23                # groups per core
NODES_PER_CORE = N_GROUPS * GROUP_NODES   # 376832
TILES_PER_GROUP = 728        # 128-edge tiles per group (capacity 93184 edges)
EDGES_PER_GROUP = TILES_PER_GROUP * 128
N_FEAT = 5


def build_nc(n_groups=N_GROUPS, tiles_per_group=TILES_PER_GROUP, mask_ring=8,
             pe_check=4, repeat=1):
    """Build the per-core Bass program (same program on all 8 cores).

    Raw-bass (no Tile): this container's walrus only supports one embedded
    sync-wait per compute instruction, so all synchronization is standalone
    wait_ge instructions plus one then_inc per producing instruction.
    """
    ng, tg = n_groups, tiles_per_group
    cols = ng * tg  # free-dim length of the per-core edge arrays
    R = mask_ring

    nc = bass.Bass()

    feats = nc.dram_tensor("feats", [128, cols * N_FEAT], F32, kind="ExternalInput")
    vals = nc.dram_tensor("vals", [128, cols], F32, kind="ExternalInput")
    mloc = nc.dram_tensor("mloc", [128, cols], I32, kind="ExternalInput")
    wvec = nc.dram_tensor("wvec", [128, 8], F32, kind="ExternalInput")
    iota_in = nc.dram_tensor("iota", [128, 128], BF16, kind="ExternalInput")
    out = nc.dram_tensor("out", [ng, 128, 128], F32, kind="ExternalOutput")

    from contextlib import ExitStack
    ctx = ExitStack()
    with ctx:
        iota_sb = ctx.enter_context(nc.sbuf_tensor("iota_sb", [128, 128], BF16))
        w_sb = ctx.enter_context(nc.sbuf_tensor("w_sb", [128, 8], F32))
        c127 = ctx.enter_context(nc.sbuf_tensor("c127", [128, 1], I32))
        c7 = ctx.enter_context(nc.sbuf_tensor("c7", [128, 1], I32))
        accum = ctx.enter_context(nc.sbuf_tensor("accum", [128, ng * 128], F32))
        f_all = ctx.enter_context(nc.sbuf_tensor("f_sb", [128, 2 * tg * N_FEAT], F32))
        v_all = ctx.enter_context(nc.sbuf_tensor("v_sb", [128, 2 * tg], F32))
        m_all = ctx.enter_context(nc.sbuf_tensor("m_sb", [128, 2 * tg], I32))
        d_all = ctx.enter_context(nc.sbuf_tensor("d_sb", [128, 2 * tg], F32))
        pi_all = ctx.enter_context(nc.sbuf_tensor("pi_sb", [128, 2 * tg], I32))
        ci_all = ctx.enter_context(nc.sbuf_tensor("ci_sb", [128, 2 * tg], I32))
        pf_all = ctx.enter_context(nc.sbuf_tensor("pf_sb", [128, 2 * tg], F32))
        cf_all = ctx.enter_context(nc.sbuf_tensor("cf_sb", [128, 2 * tg], F32))
        wm_all = ctx.enter_context(nc.sbuf_tensor("wm_sb", [128, R * 128], BF16))
        xm_all = ctx.enter_context(nc.sbuf_tensor("xm_sb", [128, R * 128], BF16))
        f_sb = [f_all[:, i * tg * N_FEAT : (i + 1) * tg * N_FEAT] for i in range(2)]
        v_sb = [v_all[:, i * tg : (i + 1) * tg] for i in range(2)]
        m_sb = [m_all[:, i * tg : (i + 1) * tg] for i in range(2)]
        d_sb = [d_all[:, i * tg : (i + 1) * tg] for i in range(2)]
        pi_sb = [pi_all[:, i * tg : (i + 1) * tg] for i in range(2)]
        ci_sb = [ci_all[:, i * tg : (i + 1) * tg] for i in range(2)]
        pf_sb = [pf_all[:, i * tg : (i + 1) * tg] for i in range(2)]
        cf_sb = [cf_all[:, i * tg : (i + 1) * tg] for i in range(2)]
        wm_sb = [wm_all[:, i * 128 : (i + 1) * 128] for i in range(R)]
        xm_sb = [xm_all[:, i * 128 : (i + 1) * 128] for i in range(R)]
        psum0 = ctx.enter_context(nc.psum_tensor("psum0", [128, 128], F32))
        psum1 = ctx.enter_context(nc.psum_tensor("psum1", [128, 128], F32))
        s_din = ctx.enter_context(nc.semaphore("s_din"))
        s_prep = ctx.enter_context(nc.semaphore("s_prep"))
        s_mask = ctx.enter_context(nc.semaphore("s_mask"))
        s_pe = ctx.enter_context(nc.semaphore("s_pe"))
        s_evict = ctx.enter_context(nc.semaphore("s_evict"))
        s_dout = ctx.enter_context(nc.semaphore("s_dout"))
        block = ctx.enter_context(nc.Block())

        psums = [psum0, psum1]
        PREP_OPS = 11  # DVE prep ops per group (must match the vector block)

        def prep_end(g):
            return 2 + PREP_OPS * (g + 1)

        @block.sync
        def _(sync):
            sync.dma_start(out=iota_sb[:], in_=iota_in[:]).then_inc(s_din, 16)
            sync.dma_start(out=w_sb[:], in_=wvec[:]).then_inc(s_din, 16)
            for rep in range(repeat):
                for g in range(ng):
                    G = rep * ng + g
                    s = G % 2
                    if G >= 2:
                        # slot tenants from G-2 fully consumed after its prep
                        sync.wait_ge(s_prep, prep_end(G - 2))
                    sync.dma_start(
                        out=f_sb[s],
                        in_=feats[:, g * tg * N_FEAT : (g + 1) * tg * N_FEAT],
                    ).then_inc(s_din, 16)
                    sync.dma_start(
                        out=v_sb[s], in_=vals[:, g * tg : (g + 1) * tg]
                    ).then_inc(s_din, 16)
                    sync.dma_start(
                        out=m_sb[s], in_=mloc[:, g * tg : (g + 1) * tg]
                    ).then_inc(s_din, 16)
            sync.wait_ge(s_evict, ng * repeat)
            out_ap = bass.AP(out, 0, [[128, 128], [128 * 128, ng], [1, 128]])
            sync.dma_start(
                out=out_ap, in_=accum[:].rearrange("p (g q) -> p g q", g=ng)
            ).then_inc(s_dout, 16)
            sync.wait_ge(s_dout, 16)

        @block.vector
        def _(vector):
            # s_prep counts DVE prep-op completions (write-visibility guard:
            # a DVE op's writes are only guaranteed visible to a later DVE op
            # after a semaphore wait on the producer's completion).
            pcnt = 0

            def V(inst):
                nonlocal pcnt
                inst.then_inc(s_prep, 1)
                pcnt += 1

            def W():
                vector.wait_ge(s_prep, pcnt)

            V(nc.vector.memset(c127[:], 127))
            V(nc.vector.memset(c7[:], 7))
            vector.wait_ge(s_din, 32)  # iota + w
            for G in range(ng * repeat):
                g = G % ng
                s = G % 2
                vector.wait_ge(s_din, 32 + 48 * (G + 1))  # f,v,m of group g
                fg = f_sb[s]
                # d = values * sum_f w_f * feat_f
                V(nc.vector.tensor_copy(d_sb[s], fg[:, 0::N_FEAT]))
                W()
                V(nc.vector.tensor_tensor(
                    out=d_sb[s],
                    in0=d_sb[s],
                    in1=w_sb[:, 0:1].to_broadcast([128, tg]),
                    op=mybir.AluOpType.mult,
                ))
                for f in range(1, N_FEAT):
                    W()
                    V(nc.vector.scalar_tensor_tensor(
                        out=d_sb[s],
                        in0=fg[:, f::N_FEAT],
                        scalar=w_sb[:, f : f + 1],
                        in1=d_sb[s],
                        op0=mybir.AluOpType.mult,
                        op1=mybir.AluOpType.add,
                    ))
                W()
                V(nc.vector.tensor_tensor(
                    out=d_sb[s], in0=d_sb[s], in1=v_sb[s], op=mybir.AluOpType.mult
                ))
                # p = m & 127, c = m >> 7, as f32
                V(nc.vector.tensor_tensor(
                    out=pi_sb[s],
                    in0=m_sb[s],
                    in1=c127[:].to_broadcast([128, tg]),
                    op=mybir.AluOpType.bitwise_and,
                ))
                V(nc.vector.tensor_tensor(
                    out=ci_sb[s],
                    in0=m_sb[s],
                    in1=c7[:].to_broadcast([128, tg]),
                    op=mybir.AluOpType.logical_shift_right,
                ))
                W()
                V(nc.vector.tensor_copy(pf_sb[s], pi_sb[s]))
                V(nc.vector.tensor_copy(cf_sb[s], ci_sb[s]))
                assert pcnt == prep_end(G), (pcnt, G)
                W()  # all prep writes visible before the mask loop reads them
                for t in range(tg):
                    gt = G * tg + t  # global tile index
                    if gt >= R and t % pe_check == 0:
                        # mask ring slots for [gt, gt+pe_check) need matmuls
                        # up to gt - R + pe_check - 1 retired
                        vector.wait_ge(s_pe, gt - R + pe_check)
                    r = gt % R
                    nc.vector.tensor_scalar(
                        wm_sb[r],
                        iota_sb[:],
                        pf_sb[s][:, t : t + 1],
                        d_sb[s][:, t : t + 1],
                        mybir.AluOpType.is_equal,
                        mybir.AluOpType.mult,
                    )
                    nc.vector.tensor_scalar(
                        xm_sb[r],
                        iota_sb[:],
                        cf_sb[s][:, t : t + 1],
                        None,
                        mybir.AluOpType.is_equal,
                    ).then_inc(s_mask, 1)
                vector.wait_ge(s_pe, (G + 1) * tg)
                nc.vector.tensor_copy(
                    accum[:, g * 128 : (g + 1) * 128], psums[s][:]
                ).then_inc(s_evict, 1)

        @block.tensor
        def _(tensor):
            for G in range(ng * repeat):
                s = G % 2
                if G >= 2:
                    tensor.wait_ge(s_evict, G - 1)  # psum slot free
                for t in range(tg):
                    gt = G * tg + t
                    tensor.wait_ge(s_mask, gt + 1)
                    r = gt % R
                    nc.tensor.matmul(
                        out=psums[s][:],
                        lhsT=xm_sb[r],
                        rhs=wm_sb[r],
                        start=(t == 0),
                        stop=(t == tg - 1),
                    ).then_inc(s_pe, 1)

    return nc


# ---------------------------------------------------------------------------
# Host-side sharding / unsharding
# ---------------------------------------------------------------------------

def prepare_in_maps(features, values, rows, n_groups=N_GROUPS,
                    tiles_per_group=TILES_PER_GROUP):
    """Bucket edges by destination node into 8 cores x n_groups groups and
    lay each group out column-major in [128, tiles] tiles."""
    ng, tg = n_groups, tiles_per_group
    cols = ng * tg
    epg = tg * 128
    total_groups = N_CORES * ng

    features = np.asarray(features, dtype=np.float32)
    values = np.asarray(values, dtype=np.float32)
    rows = np.asarray(rows, dtype=np.int32)

    g_global = rows // GROUP_NODES  # [E] in [0, total_groups)
    order = np.argsort(g_global, kind="stable")
    g_sorted = g_global[order]
    counts = np.bincount(g_sorted, minlength=total_groups)
    if counts.max() > epg:
        raise RuntimeError(
            f"group overflow: max edges per group {counts.max()} > capacity {epg}"
        )
    starts = np.zeros(total_groups, dtype=np.int64)
    starts[1:] = np.cumsum(counts)[:-1]

    # destination flat position inside the owning core's [128, cols] array
    j_within = np.arange(len(rows), dtype=np.int64) - starts[g_sorted]
    g_local = (g_sorted % ng).astype(np.int64)
    pos = (j_within % 128) * cols + g_local * tg + (j_within // 128)
    core_of = (g_sorted // ng).astype(np.int64)
    gpos = core_of * (128 * cols) + pos  # position in a [8, 128, cols] array

    def scatter(src_sorted, fill=0.0, dtype=np.float32):
        dst = np.full(N_CORES * 128 * cols, fill, dtype=dtype)
        dst[gpos] = src_sorted
        return dst.reshape(N_CORES, 128, cols)

    vals_all = scatter(values[order])
    mloc_all = scatter((rows[order] - g_sorted * GROUP_NODES).astype(np.int32),
                       fill=0, dtype=np.int32)
    feats_flat = np.zeros((N_CORES * 128 * cols, N_FEAT), dtype=np.float32)
    feats_flat[gpos] = features[order]
    feats_all = feats_flat.reshape(N_CORES, 128, cols * N_FEAT)

    w8 = np.zeros(8, dtype=np.float32)
    return vals_all, mloc_all, feats_all, w8


def make_in_maps(features, values, a0_weight, rows,
                 n_groups=N_GROUPS, tiles_per_group=TILES_PER_GROUP):
    vals_all, mloc_all, feats_all, w8 = prepare_in_maps(
        features, values, rows, n_groups, tiles_per_group)
    w8[:N_FEAT] = np.asarray(a0_weight, dtype=np.float32).reshape(-1)[:N_FEAT]
    wvec = np.tile(w8[None, :], (128, 1)).astype(np.float32)
    iota = np.tile(np.arange(128, dtype=np.float32)[None, :], (128, 1)).astype(
        ml_dtypes.bfloat16
    )
    in_maps = []
    for c in range(N_CORES):
        in_maps.append({
            "feats": np.ascontiguousarray(feats_all[c]),
            "vals": np.ascontiguousarray(vals_all[c]),
            "mloc": np.ascontiguousarray(mloc_all[c]),
            "wvec": wvec,
            "iota": iota,
        })
    return in_maps


def timed_run(nc, in_maps, iters=5):
    """Run the kernel via PJRT with device-resident inputs and time executes.

    Returns (results_list, best_seconds). Wall-clock includes the axon RPC
    dispatch, so the min over iters is an upper bound on HW time.
    """
    import time
    import jax
    import concourse.mybir as _mybir
    from jax.sharding import Mesh, PartitionSpec, NamedSharding
    from jax.experimental.shard_map import shard_map
    from concourse import bass2jax as b2j

    b2j.install_neuronx_cc_hook()
    n_cores = len(in_maps)
    partition_name = nc.partition_id_tensor.name if nc.partition_id_tensor else None

    in_names, out_names, out_avals, zero_outs = [], [], [], []
    for alloc in nc.m.functions[0].allocations:
        if not isinstance(alloc, _mybir.MemoryLocationSet):
            continue
        name = alloc.memorylocations[0].name
        if alloc.kind == "ExternalInput":
            if name != partition_name:
                in_names.append(name)
        elif alloc.kind == "ExternalOutput":
            shape = tuple(alloc.tensor_shape)
            dtype = _mybir.dt.np(alloc.dtype)
            out_names.append(name)
            out_avals.append(jax.core.ShapedArray(shape, dtype))
            zero_outs.append(np.zeros(shape, dtype))
    n_params = len(in_names)
    all_in_names = list(in_names) + list(out_names)
    if partition_name is not None:
        all_in_names.append(partition_name)

    def _body(*args):
        operands = list(args)
        if partition_name is not None:
            operands.append(b2j.partition_id_tensor())
        outs = b2j._bass_exec_p.bind(
            *operands,
            out_avals=tuple(out_avals),
            in_names=tuple(all_in_names),
            out_names=tuple(out_names),
            lowering_input_output_aliases=(),
            sim_require_finite=True,
            sim_require_nnan=True,
            nc=nc,
        )
        return tuple(outs)

    devices = jax.devices()[:n_cores]
    mesh = Mesh(np.asarray(devices), ("core",))
    n_ops = n_params + len(out_names)
    fn = jax.jit(
        shard_map(
            _body,
            mesh=mesh,
            in_specs=(PartitionSpec("core"),) * n_ops,
            out_specs=(PartitionSpec("core"),) * len(out_names),
            check_rep=False,
        ),
        keep_unused=True,
    )
    concat_in = [
        np.concatenate([np.asarray(in_maps[c][nm]) for c in range(n_cores)], axis=0)
        for nm in in_names
    ]
    concat_zero = [
        np.zeros((n_cores * z.shape[0], *z.shape[1:]), z.dtype) for z in zero_outs
    ]
    sh = NamedSharding(mesh, PartitionSpec("core"))
    dev_args = [jax.device_put(x, sh) for x in concat_in + concat_zero]
    outs = fn(*dev_args)
    jax.block_until_ready(outs)
    best = float("inf")
    for _ in range(iters):
        t0 = time.perf_counter()
        outs = fn(*dev_args)
        jax.block_until_ready(outs)
        best = min(best, time.perf_counter() - t0)
    results = [
        {
            nm: np.asarray(outs[i]).reshape(n_cores, *out_avals[i].shape)[c]
            for i, nm in enumerate(out_names)
        }
        for c in range(n_cores)
    ]
    return results, best


_CACHE = {}


def kernel(features, values, a0_weight, rows, num_nodes):
    assert int(num_nodes) == NUM_NODES
    in_maps = make_in_maps(features, values, a0_weight, rows)
    if "nc" not in _CACHE:
        _CACHE["nc"] = build_nc()
    nc = _CACHE["nc"]
    res = run_bass_kernel_spmd(nc, in_maps, core_ids=list(range(N_CORES)))
    outs = [r["out"].reshape(-1) for r in res.results]
    full = np.concatenate(outs)[:NUM_NODES]
    return full.astype(np.float32)



# revision 4
# speedup vs baseline: 51.3220x; 51.3220x over previous
"""Trainium2 Bass kernel for BERT4ETH adjacency build:
    data = values * (features @ a0_weight[0])        # [E]
    out  = segment_sum(data, rows, num_segments=3M)  # [3M]

Strategy (8 NeuronCores): the host routes each edge to the core that owns
its destination node and lays the edges of every node out CONTIGUOUSLY in
fixed-width "degree class" regions (one class per distinct node degree, so
there is zero slot padding).  On device the whole problem then becomes a
dense streaming computation with NO scatter at all:

    per chunk (one degree class d, K node-columns x 128 partitions):
      t = sum_f w_f * F_f          (5 DVE ops over [128, K*d], bf16)
      m = t * V                    (1 DVE op)
      out[:, cols] = reduce_add(m.view(128, K, d), axis=-1)   (1 DVE op)

Every node owns d consecutive slots of one partition row, so the
segment-sum is a fixed-width innermost-axis tensor_reduce.  Output ranges
are disjoint across cores -> no all-reduce; the host inverse-permutes the
[8, 128, OUT_COLS] result back to the full [3M] vector.

All edge payloads ship as bf16 (measured end-to-end l2 rel err ~4e-3 vs
the 2e-2 gate); accumulation happens in f32 inside tensor_reduce.
"""

import numpy as np
import ml_dtypes

import concourse.bass as bass
import concourse.mybir as mybir
from concourse.bass_utils import run_bass_kernel_spmd

F32 = mybir.dt.float32
BF16 = mybir.dt.bfloat16

N_CORES = 8
NUM_NODES = 3_000_000
N_FEAT = 5
CHUNK_SLOTCOLS = 3072   # max slot-columns (per plane) handled by one chunk
OPS_PER_CHUNK = 7       # 5 feature ops + 1 value mult + 1 reduce
BF = ml_dtypes.bfloat16


# ---------------------------------------------------------------------------
# Host-side layout: degree-class packing
# ---------------------------------------------------------------------------

class Layout:
    """Host plan: where every node/edge lives in the per-core streams."""

    def __init__(self, rows, num_nodes):
        rows = np.asarray(rows)
        E = rows.shape[0]
        counts = np.bincount(rows, minlength=num_nodes)
        order = np.argsort(rows, kind="stable")
        starts = np.zeros(num_nodes + 1, np.int64)
        np.cumsum(counts, out=starts[1:])
        rs = rows[order].astype(np.int64)
        slot = np.arange(E, dtype=np.int64) - starts[rs]

        maxdeg = int(counts.max())
        node_core = np.full(num_nodes, -1, np.int32)
        node_part = np.zeros(num_nodes, np.int32)
        node_base = np.zeros(num_nodes, np.int64)   # chunk-block start in stream
        node_L = np.zeros(num_nodes, np.int32)      # slot-cols per plane of chunk
        node_colw = np.zeros(num_nodes, np.int32)   # col_in_chunk * d
        node_outcol = np.zeros(num_nodes, np.int32)

        chunks = []      # (L, K, d, out_off) in stream order
        stream_off = 0   # in columns of the [128, TOTAL] bf16 stream
        out_off = 0
        for d in range(1, maxdeg + 1):
            ids = np.flatnonzero(counts == d)
            n_d = ids.size
            if n_d == 0:
                continue
            npc = -(-n_d // N_CORES)        # nodes per core (ceil)
            cols = -(-npc // 128)           # node-cols per core
            idx = np.arange(n_d, dtype=np.int64)
            core = (idx % N_CORES).astype(np.int32)
            pos = idx // N_CORES
            part = (pos % 128).astype(np.int32)
            nodecol = pos // 128
            kmax = max(1, CHUNK_SLOTCOLS // d)
            nchunks = -(-cols // kmax)
            ks = [min(kmax, cols - i * kmax) for i in range(nchunks)]
            bases = np.empty(nchunks, np.int64)
            for i, k in enumerate(ks):
                chunks.append((k * d, k, d, out_off + i * kmax))
                bases[i] = stream_off
                stream_off += 6 * k * d
            cid = nodecol // kmax
            node_core[ids] = core
            node_part[ids] = part
            node_base[ids] = bases[cid]
            node_L[ids] = np.asarray([k * d for k in ks], np.int32)[cid]
            node_colw[ids] = ((nodecol % kmax) * d).astype(np.int32)
            node_outcol[ids] = (out_off + nodecol).astype(np.int32)
            out_off += cols

        self.chunks = tuple(chunks)
        self.total_cols = stream_off
        self.out_cols = out_off
        self.counts = counts
        self.order = order
        # per sorted-edge destination (flat index into [8*128, TOTAL])
        e_row = node_core[rs] * 128 + node_part[rs]
        self.e_flat = e_row.astype(np.int64) * stream_off + (
            node_base[rs] + node_colw[rs] + slot
        )
        self.e_L = node_L[rs].astype(np.int64)
        self.node_core = node_core
        self.node_part = node_part
        self.node_outcol = node_outcol

    def build_streams(self, features, values):
        feats_s = np.asarray(features)[self.order].astype(BF)   # [E,5]
        vals_s = np.asarray(values)[self.order].astype(BF)
        flat = np.zeros(N_CORES * 128 * self.total_cols, BF)
        for f in range(N_FEAT):
            flat[self.e_flat + f * self.e_L] = feats_s[:, f]
        flat[self.e_flat + N_FEAT * self.e_L] = vals_s
        return flat.reshape(N_CORES, 128, self.total_cols)

    def gather_output(self, outs, num_nodes):
        """outs: [8, 128, OUT_COLS] f32 -> full [num_nodes] f32."""
        full = np.zeros(num_nodes, np.float32)
        mask = self.counts > 0
        full[mask] = outs[
            self.node_core[mask], self.node_part[mask], self.node_outcol[mask]
        ]
        return full


def make_in_maps(features, values, a0_weight, rows, num_nodes=NUM_NODES):
    lay = Layout(rows, num_nodes)
    streams = lay.build_streams(features, values)
    w8 = np.zeros(8, np.float32)
    w8[:N_FEAT] = np.asarray(a0_weight, np.float32).reshape(-1)[:N_FEAT]
    wvec = np.tile(w8[None, :], (128, 1)).astype(np.float32)
    in_maps = [
        {"stream": np.ascontiguousarray(streams[c]), "wvec": wvec}
        for c in range(N_CORES)
    ]
    return lay, in_maps


# ---------------------------------------------------------------------------
# Device program
# ---------------------------------------------------------------------------

def build_nc(chunks, total_cols, out_cols, repeat=1):
    """Per-core Bass program (same on all 8 cores).

    Raw bass: standalone wait_ge instructions + one then_inc per producer
    (this container's walrus supports one embedded sync-wait per compute
    instruction; standalone waits keep it simple and cheap at this scale).
    """
    nc = bass.Bass()
    stream = nc.dram_tensor("stream", [128, total_cols], BF16, kind="ExternalInput")
    wvec = nc.dram_tensor("wvec", [128, 8], F32, kind="ExternalInput")
    out = nc.dram_tensor("out", [128, out_cols], F32, kind="ExternalOutput")

    lmax = max(L for (L, K, d, o) in chunks)
    C = len(chunks)
    OPS = OPS_PER_CHUNK
    soff = []
    acc = 0
    for (L, K, d, o) in chunks:
        soff.append(acc)
        acc += 6 * L
    assert acc == total_cols

    from contextlib import ExitStack
    ctx = ExitStack()
    with ctx:
        w_sb = ctx.enter_context(nc.sbuf_tensor("w_sb", [128, 8], F32))
        bufs = ctx.enter_context(nc.sbuf_tensor("bufs", [128, 2 * 6 * lmax], BF16))
        buf = [bufs[:, i * 6 * lmax : (i + 1) * 6 * lmax] for i in range(2)]
        t_sb = ctx.enter_context(nc.sbuf_tensor("t_sb", [128, lmax], BF16))
        m_sb = ctx.enter_context(nc.sbuf_tensor("m_sb", [128, lmax], BF16))
        out_sb = ctx.enter_context(nc.sbuf_tensor("out_sb", [128, out_cols], F32))
        s_w = ctx.enter_context(nc.semaphore("s_w"))
        s_d0 = ctx.enter_context(nc.semaphore("s_d0"))
        s_d1 = ctx.enter_context(nc.semaphore("s_d1"))
        s_cmp = ctx.enter_context(nc.semaphore("s_cmp"))
        s_dout = ctx.enter_context(nc.semaphore("s_dout"))
        s_db = [s_d0, s_d1]
        block = ctx.enter_context(nc.Block())

        @block.sync
        def _(sync):
            sync.dma_start(out=w_sb[:], in_=wvec[:]).then_inc(s_w, 16)
            for rep in range(repeat):
                for i, (L, K, d, o) in enumerate(chunks):
                    t = rep * C + i
                    if t >= 2:
                        # buffer t%2 fully consumed once chunk t-2 retired
                        sync.wait_ge(s_cmp, OPS * (t - 1))
                    sync.dma_start(
                        out=buf[t % 2][:, : 6 * L],
                        in_=stream[:, soff[i] : soff[i] + 6 * L],
                    ).then_inc(s_db[t % 2], 16)
            sync.wait_ge(s_cmp, OPS * C * repeat)
            sync.dma_start(out=out[:], in_=out_sb[:]).then_inc(s_dout, 16)
            sync.wait_ge(s_dout, 16)

        @block.vector
        def _(vector):
            # s_cmp counts DVE op completions (write-visibility guard between
            # dependent DVE ops, and the DMA double-buffer handshake).
            pcnt = 0

            def V(inst):
                nonlocal pcnt
                inst.then_inc(s_cmp, 1)
                pcnt += 1

            def W():
                vector.wait_ge(s_cmp, pcnt)

            vector.wait_ge(s_w, 16)
            for rep in range(repeat):
                for i, (L, K, d, o) in enumerate(chunks):
                    t = rep * C + i
                    vector.wait_ge(s_db[t % 2], 16 * (t // 2 + 1))
                    b = buf[t % 2]
                    F = [b[:, f * L : (f + 1) * L] for f in range(6)]
                    tt = t_sb[:, :L]
                    W()  # all prior-chunk ops retired (t_sb/m_sb reuse)
                    V(nc.vector.tensor_scalar(
                        tt, F[0], w_sb[:, 0:1], None, mybir.AluOpType.mult
                    ))
                    for f in range(1, N_FEAT):
                        W()
                        V(nc.vector.scalar_tensor_tensor(
                            out=tt,
                            in0=F[f],
                            scalar=w_sb[:, f : f + 1],
                            in1=tt,
                            op0=mybir.AluOpType.mult,
                            op1=mybir.AluOpType.add,
                        ))
                    W()
                    V(nc.vector.tensor_tensor(
                        out=m_sb[:, :L], in0=tt, in1=F[5], op=mybir.AluOpType.mult
                    ))
                    W()
                    V(nc.vector.tensor_reduce(
                        out=out_sb[:, o : o + K],
                        in_=m_sb[:, :L].rearrange("p (k d) -> p k d", d=d),
                        axis=mybir.AxisListType.X,
                        op=mybir.AluOpType.add,
                    ))

    return nc


# ---------------------------------------------------------------------------
# Timed run helper (PJRT, device-resident inputs; used by test.py only)
# ---------------------------------------------------------------------------

def timed_run(nc, in_maps, iters=5):
    import time
    import jax
    import concourse.mybir as _mybir
    from jax.sharding import Mesh, PartitionSpec, NamedSharding
    from jax.experimental.shard_map import shard_map
    from concourse import bass2jax as b2j

    b2j.install_neuronx_cc_hook()
    n_cores = len(in_maps)
    partition_name = nc.partition_id_tensor.name if nc.partition_id_tensor else None

    in_names, out_names, out_avals, zero_outs = [], [], [], []
    for alloc in nc.m.functions[0].allocations:
        if not isinstance(alloc, _mybir.MemoryLocationSet):
            continue
        name = alloc.memorylocations[0].name
        if alloc.kind == "ExternalInput":
            if name != partition_name:
                in_names.append(name)
        elif alloc.kind == "ExternalOutput":
            shape = tuple(alloc.tensor_shape)
            dtype = _mybir.dt.np(alloc.dtype)
            out_names.append(name)
            out_avals.append(jax.core.ShapedArray(shape, dtype))
            zero_outs.append(np.zeros(shape, dtype))
    n_params = len(in_names)
    all_in_names = list(in_names) + list(out_names)
    if partition_name is not None:
        all_in_names.append(partition_name)

    def _body(*args):
        operands = list(args)
        if partition_name is not None:
            operands.append(b2j.partition_id_tensor())
        outs = b2j._bass_exec_p.bind(
            *operands,
            out_avals=tuple(out_avals),
            in_names=tuple(all_in_names),
            out_names=tuple(out_names),
            lowering_input_output_aliases=(),
            sim_require_finite=True,
            sim_require_nnan=True,
            nc=nc,
        )
        return tuple(outs)

    devices = jax.devices()[:n_cores]
    mesh = Mesh(np.asarray(devices), ("core",))
    n_ops = n_params + len(out_names)
    fn = jax.jit(
        shard_map(
            _body,
            mesh=mesh,
            in_specs=(PartitionSpec("core"),) * n_ops,
            out_specs=(PartitionSpec("core"),) * len(out_names),
            check_rep=False,
        ),
        keep_unused=True,
    )
    concat_in = [
        np.concatenate([np.asarray(in_maps[c][nm]) for c in range(n_cores)], axis=0)
        for nm in in_names
    ]
    concat_zero = [
        np.zeros((n_cores * z.shape[0], *z.shape[1:]), z.dtype) for z in zero_outs
    ]
    sh = NamedSharding(mesh, PartitionSpec("core"))
    dev_args = [jax.device_put(x, sh) for x in concat_in + concat_zero]
    outs = fn(*dev_args)
    jax.block_until_ready(outs)
    best = float("inf")
    for _ in range(iters):
        t0 = time.perf_counter()
        outs = fn(*dev_args)
        jax.block_until_ready(outs)
        best = min(best, time.perf_counter() - t0)
    results = [
        {
            nm: np.asarray(outs[i]).reshape(n_cores, *out_avals[i].shape)[c]
            for i, nm in enumerate(out_names)
        }
        for c in range(n_cores)
    ]
    return results, best


# ---------------------------------------------------------------------------
# Entry point
# ---------------------------------------------------------------------------

_CACHE = {}


def kernel(features, values, a0_weight, rows, num_nodes):
    num_nodes = int(num_nodes)
    lay, in_maps = make_in_maps(features, values, a0_weight, rows, num_nodes)
    key = (lay.chunks, lay.total_cols, lay.out_cols)
    if key not in _CACHE:
        _CACHE[key] = build_nc(lay.chunks, lay.total_cols, lay.out_cols)
    nc = _CACHE[key]
    res = run_bass_kernel_spmd(nc, in_maps, core_ids=list(range(N_CORES)))
    outs = np.stack([r["out"] for r in res.results])  # [8,128,OUT_COLS]
    return lay.gather_output(outs, num_nodes)
